# revision 19
# baseline (speedup 1.0000x reference)
"""DiKT (DKVMN-style knowledge tracing) Trainium2 kernel.

Self-contained: builds a Bass/Tile program, shards batch over 8 NeuronCores
(pure data parallel, 16 batch rows per core), runs via run_bass_kernel_spmd.

Algorithm per core (B_loc=16, V=128, C=64, S=128 steps):
  Both value memories (right/wrong) live as ONE SBUF tensor m[v=128, col=2048]
  with col = r*64 + c, r = mem*16 + b.  Per step:
     m' = m * (1 - e x w) + a x w

Two consecutive steps are FUSED into one update (64 pairs):
     Shat = S1*S2 = 1 - e1 x w1 - e2 x w2 + (e1e2) x (w1w2)
     Ahat = S2*A1 + A2 = a1 x w1 - (a1e2) x (w1w2) + a2 x w2
Both are rank-3 sums of outer products, built by ONE K=96 TensorE matmul
against a 3-group block-diagonal rhs (per group g, row r: the (r,c) diagonal
block carries [-w1 | -w2 | +w1w2]).  lhsT groups: [e1 | e2 | e1e2] for Shat,
[-a1 | -a2 | -a1e2] for Ahat (signs make every product come out right).

Consumption per pair uses PSUM-resident accumulation (validated on HW):
  E-matmul writes Shat-1 into a PSUM bank; a DVE scalar_tensor_tensor runs
  IN-PLACE on the bank: bank = (bank + 1) * m; the A-matmul then
  accumulates Ahat onto the same bank (start=False, skip_group_check); one
  1024-col ACT copy moves the finished columns back to fp16 m.
  This costs 1 DVE op + 1 copy per 512-col chunk instead of the previous
  2 elementwise ops + 2 ACT evacuations.

The TensorE p-state ramp (2.4GHz only after ~3us of gapless execution, else
1.2GHz) is held at full clock by filler matmuls into a scratch bank whenever
the real per-pair matmul work would leave gaps.

The pre-phase (embedding gathers, transposes, e/a/w computation, and the
block-diagonal wd scatter to DRAM) is emitted interleaved with the pair loop
in per-4-pair granules so it executes in the other engines' slack instead of
serializing in front of the loop.
"""

import numpy as np

import concourse.mybir as mybir
from concourse import bass, bacc, tile
from concourse.bass_utils import run_bass_kernel_spmd

F16 = mybir.dt.float16
F32 = mybir.dt.float32
I32 = mybir.dt.int32
ALU = mybir.AluOpType
ACT = mybir.ActivationFunctionType
AX = mybir.AxisListType

# model dims
KD = 128      # KEY_DIM
VD = 128      # VALUE_DIM
SD = 128      # SUMMARY_DIM
Q = 10000     # QUESTION_NUM
C = 64        # CONCEPT_NUM
B = 128       # full batch
S = 128       # seq len
NCORE = 8
BL = B // NCORE          # 16 batch rows per core
NR = 2 * BL              # 32 rows per step (right+wrong)
COLS = NR * C            # 2048 memory columns per core
NP = S // 2              # 64 step pairs
KF = 3 * NR              # 96 contraction rows per fused matmul
PB = KF * COLS           # 196608 elements per pair block in wd_dram
NGQ = (S * NR) // 128    # 32 q-side gather chunks of 128 rows
NPG = NP // 4            # 16 e/a matmul groups of 4 pairs
NDUM = 1                 # filler ldweights per pair for the PE p-state hold
RING = 8                 # e16 staging depth in pairs
PF = 6                   # E-matmul prefetch distance in pairs

import os
DEBUG = bool(os.environ.get("KDEBUG"))


def _build_program():
    nc = bacc.Bacc(trn_type="TRN2", target_bir_lowering=False, num_devices=NCORE,
                   num_swdge_queues=4)

    # ---- DRAM inputs ----
    i_emb = nc.dram_tensor("i_emb", [2 * Q + 1, VD], F16, kind="ExternalInput")
    q_emb = nc.dram_tensor("q_emb", [Q + 1, KD], F16, kind="ExternalInput")
    idx_i = nc.dram_tensor("idx_i", [128, NGQ], I32, kind="ExternalInput")
    idx_q = nc.dram_tensor("idx_q", [128, NGQ], I32, kind="ExternalInput")
    idx_t = nc.dram_tensor("idx_t", [BL, 1], I32, kind="ExternalInput")

    erase_Wt = nc.dram_tensor("erase_Wt", [VD, VD], F16, kind="ExternalInput")
    add_Wt = nc.dram_tensor("add_Wt", [VD, VD], F16, kind="ExternalInput")
    key_Wt = nc.dram_tensor("key_Wt", [KD, C], F16, kind="ExternalInput")
    erase_b_row = nc.dram_tensor("erase_b_row", [1, 4 * VD], F16, kind="ExternalInput")
    add_b_row = nc.dram_tensor("add_b_row", [1, 4 * VD], F16, kind="ExternalInput")
    rsum_Wt0 = nc.dram_tensor("rsum_Wt0", [VD, SD], F16, kind="ExternalInput")
    rsum_Wt1 = nc.dram_tensor("rsum_Wt1", [KD, SD], F16, kind="ExternalInput")
    wsum_Wt0 = nc.dram_tensor("wsum_Wt0", [VD, SD], F16, kind="ExternalInput")
    wsum_Wt1 = nc.dram_tensor("wsum_Wt1", [KD, SD], F16, kind="ExternalInput")
    rsum_b_col = nc.dram_tensor("rsum_b_col", [SD, 1], F32, kind="ExternalInput")
    wsum_b_col = nc.dram_tensor("wsum_b_col", [SD, 1], F32, kind="ExternalInput")
    succ_Wt = nc.dram_tensor("succ_Wt", [SD, 1], F16, kind="ExternalInput")
    fail_Wt = nc.dram_tensor("fail_Wt", [SD, 1], F16, kind="ExternalInput")
    diff_Wt = nc.dram_tensor("diff_Wt", [KD, 1], F16, kind="ExternalInput")
    succ_b = nc.dram_tensor("succ_b", [1, 1], F32, kind="ExternalInput")
    fail_b = nc.dram_tensor("fail_b", [1, 1], F32, kind="ExternalInput")
    diff_b = nc.dram_tensor("diff_b", [1, 1], F32, kind="ExternalInput")
    rmem0 = nc.dram_tensor("rmem0", [VD, C], F16, kind="ExternalInput")
    wmem0 = nc.dram_tensor("wmem0", [VD, C], F16, kind="ExternalInput")
    ones_row = nc.dram_tensor("ones_row", [1, 128], F16, kind="ExternalInput")
    ones_col32 = nc.dram_tensor("ones_col32", [128, 1], F32, kind="ExternalInput")
    id128 = nc.dram_tensor("id128", [128, 128], F16, kind="ExternalInput")
    right_full = nc.dram_tensor("right_full", [B, S], I32, kind="ExternalInput")
    wrong_full = nc.dram_tensor("wrong_full", [B, S], I32, kind="ExternalInput")

    out_d = nc.dram_tensor("out", [BL, 1], F32, kind="ExternalOutput")
    if DEBUG:
        dbg_m = nc.dram_tensor("dbg_m", [VD, COLS], F16, kind="ExternalOutput")
        dbg_rr = nc.dram_tensor("dbg_rr", [VD, NR], F32, kind="ExternalOutput")

    # fused block-diagonal rhs for every pair, flat fp16:
    # pair p, group g (0..2), row r (0..31): diag block at
    #   p*PB + g*65536 + r*2112, 64 wide
    wd_dram = nc.dram_tensor("wd_dram", [NP * PB], F16, kind="ExternalInput")

    # ---- persistent SBUF ----
    sb = lambda name, shape, dt: nc.alloc_sbuf_tensor(name, shape, dt)
    m0b = sb("m0b", [VD, COLS], F16)          # broadcast initial memory
    # staged Shat-1 per pair: one tensor per ring slot (dep isolation)
    e16s = [sb(f"e16_{s}", [128, COLS], F16) for s in range(RING)]
    vecT = sb("vecT", [128, NP * 64], F16)   # i_emb rows [t1|t2] per pair, transposed
    qT = sb("qT", [128, NGQ * 128], F16)
    eP = sb("eP", [KF, NP * 128], F16)       # [e1 | e2 | e1e2] per pair
    naP = sb("naP", [KF, NP * 128], F16)     # [-a1 | -a2 | -a1e2] per pair
    scrB = sb("scrB", [KF, NP * 128], F16)   # e2 staging at group-2 partitions
    w_all = sb("w_all", [128, NGQ * C], F16)  # -w per step (4 steps/chunk)
    wstB = sb("wstB", [KF, NGQ * C], F16)    # -w2 at w1 partitions
    wprodB = sb("wprodB", [KF, NGQ * C], F16)  # w1*w2
    w_eWt = sb("w_eWt", [VD, VD], F16)
    w_aWt = sb("w_aWt", [VD, VD], F16)
    w_kWt = sb("w_kWt", [KD, C], F16)
    w_eb = sb("w_eb", [1, 4 * VD], F16)
    w_ab = sb("w_ab", [1, 4 * VD], F16)
    w_ones = sb("w_ones", [1, 128], F16)
    w_ones_c32 = sb("w_ones_c32", [128, 1], F32)
    w_id = sb("w_id", [128, 128], F16)
    idx_i_sb = sb("idx_i_sb", [128, NGQ], I32)
    idx_q_sb = sb("idx_q_sb", [128, NGQ], I32)
    idx_t_sb = sb("idx_t_sb", [BL, 1], I32)
    w_rs0 = sb("w_rs0", [VD, SD], F16)
    w_rs1 = sb("w_rs1", [KD, SD], F16)
    w_ws0 = sb("w_ws0", [VD, SD], F16)
    w_ws1 = sb("w_ws1", [KD, SD], F16)
    w_rsb = sb("w_rsb", [SD, 1], F32)
    w_wsb = sb("w_wsb", [SD, 1], F32)
    w_succ = sb("w_succ", [SD, 1], F16)
    w_fail = sb("w_fail", [SD, 1], F16)
    w_diff = sb("w_diff", [KD, 1], F16)
    w_sb_b = sb("w_sb_b", [1, 3], F32)  # succ_b, fail_b, diff_b columns 0..2
    wb_s = sb("wb_s", [128, COLS], F16)   # target corr weights, broadcast
    qvT_s = sb("qvT_s", [KD, BL], F16)    # target question emb, transposed
    sigs_s = sb("sigs_s", [1, 2], F32)    # sigmoid(success/failure counts)

    # ---- persistent PSUM: the memory itself lives here (4 banks).
    # Two independent half tensors so the dependency tracker (whole-tensor
    # granularity for persistent tensors) does not serialize the halves.
    mbs = [nc.alloc_psum_tensor(f"mb{h}", [128, 1024], F32) for h in range(2)]

    with tile.TileContext(nc) as tc:
        with tc.tile_pool(name="sbp", bufs=3) as sbp, \
             tc.tile_pool(name="psp", bufs=2, space="PSUM") as psp, \
             tc.tile_pool(name="wdp", bufs=PF + 2) as wdp:

            # ---------- constant loads ----------
            # gather/transpose-critical consts first on the sync queue; the
            # rest ride the gpsimd queue so the bootstrap gathers start
            # immediately.
            for dst, src in [
                (idx_i_sb, idx_i), (idx_q_sb, idx_q), (idx_t_sb, idx_t),
                (w_id, id128), (w_eWt, erase_Wt), (w_aWt, add_Wt),
                (w_kWt, key_Wt), (w_eb, erase_b_row), (w_ab, add_b_row),
                (w_ones, ones_row),
            ]:
                nc.sync.dma_start(out=dst[:, :], in_=src[:, :])
            for dst, src in [
                (w_ones_c32, ones_col32),
                (w_rs0, rsum_Wt0), (w_rs1, rsum_Wt1),
                (w_ws0, wsum_Wt0), (w_ws1, wsum_Wt1),
                (w_rsb, rsum_b_col), (w_wsb, wsum_b_col),
                (w_succ, succ_Wt), (w_fail, fail_Wt), (w_diff, diff_Wt),
            ]:
                nc.gpsimd.dma_start(out=dst[:, :], in_=src[:, :])
            nc.gpsimd.dma_start(out=w_sb_b[:, 0:1], in_=succ_b[:, :])
            nc.gpsimd.dma_start(out=w_sb_b[:, 1:2], in_=fail_b[:, :])
            nc.gpsimd.dma_start(out=w_sb_b[:, 2:3], in_=diff_b[:, :])

            # warm the PE clock while the boot DMAs run
            for _ in range(36):
                nc.tensor.ldweights(w_id[:, :])

            # init m: broadcast mem inits over the 16 batch blocks
            rmem_t = sbp.tile([VD, C], F16, tag="memi")
            nc.sync.dma_start(out=rmem_t[:, :], in_=rmem0[:, :])
            wmem_t = sbp.tile([VD, C], F16, tag="memi2")
            nc.sync.dma_start(out=wmem_t[:, :], in_=wmem0[:, :])
            for r in range(NR):
                srct = rmem_t if r < BL else wmem_t
                nc.vector.tensor_copy(m0b[:, r * C:(r + 1) * C], srct[:, :])
            for k in range(4):
                nc.tensor.matmul(mbs[k // 2][:, 512 * (k % 2):512 * (k % 2 + 1)],
                                 w_id[:, :], m0b[:, 512 * k:512 * (k + 1)],
                                 start=True, stop=True)

            # ---------- pre-phase emission helpers ----------
            def gather_i(g):
                """i-side gather + PE transpose for 128-row chunk g."""
                lo = g * 128
                gi16 = sbp.tile([128, VD], F16, tag="gi16")
                nc.gpsimd.indirect_dma_start(
                    out=gi16[:, :], out_offset=None,
                    in_=i_emb[:, :],
                    in_offset=bass.IndirectOffsetOnAxis(
                        ap=idx_i_sb[:, g:g + 1], axis=0),
                )
                tps = psp.tile([128, 128], F16, tag="ring")
                nc.tensor.transpose(tps[:, :], gi16[:, :], w_id[:, :])
                nc.vector.tensor_copy(vecT[:, lo:lo + 128], tps[:, :])

            def gather_q(g):
                """q-side gather + PE transpose for 128-row chunk g."""
                lo = g * 128
                gq16 = sbp.tile([128, KD], F16, tag="gq16")
                nc.gpsimd.indirect_dma_start(
                    out=gq16[:, :], out_offset=None,
                    in_=q_emb[:, :],
                    in_offset=bass.IndirectOffsetOnAxis(
                        ap=idx_q_sb[:, g:g + 1], axis=0),
                )
                tps2 = psp.tile([128, 128], F16, tag="ring")
                nc.tensor.transpose(tps2[:, :], gq16[:, :], w_id[:, :])
                nc.vector.tensor_copy(qT[:, lo:lo + 128], tps2[:, :])

            def gather_chunk(g):
                gather_i(g)
                gather_q(g)

            def soft4(g0):
                """w = -softmax(qv @ key_W.T) for chunks g0..g0+3, one Exp
                table load for the batch."""
                zq = psp.tile([128, 256], F32, tag="ring")
                for i in range(4):
                    g = g0 + i
                    nc.tensor.matmul(zq[:, 64 * i:64 * (i + 1)],
                                     qT[:, g * 128:(g + 1) * 128], w_kWt[:, :],
                                     start=True, stop=True)
                wexq = sbp.tile([128, 256], F32, tag="wexq")
                nc.scalar.activation(wexq[:, :], zq[:, :], ACT.Exp)
                smq = sbp.tile([128, 4], F32, tag="smq")
                nc.vector.tensor_reduce(
                    smq[:, :], wexq[:].rearrange("p (g c) -> p g c", c=C),
                    AX.X, ALU.add)
                rcq = sbp.tile([128, 4], F32, tag="rcq")
                nc.vector.reciprocal(rcq[:, :], smq[:, :])
                for i in range(4):
                    g = g0 + i
                    nc.gpsimd.tensor_scalar(w_all[:, g * C:(g + 1) * C],
                                            wexq[:, 64 * i:64 * (i + 1)],
                                            rcq[:, i:i + 1], -1.0,
                                            ALU.mult, ALU.mult)

            def ea_mms(j, slot):
                """e (cols 0:512) and a (cols 512:1024) matmuls for pair
                group j into one ring slot."""
                for k in range(4):
                    p = 4 * j + k
                    vl = vecT[:, 64 * p:64 * (p + 1)]
                    nc.tensor.matmul(slot[0:64, 128 * k:128 * (k + 1)],
                                     vl, w_eWt[:, :], start=True, stop=True)
                    nc.tensor.matmul(slot[0:64, 512 + 128 * k:512 + 128 * (k + 1)],
                                     vl, w_aWt[:, :], start=True, stop=True)
                nc.tensor.matmul(slot[0:64, 0:512], w_ones[:, :64], w_eb[:, :],
                                 start=False, stop=True, skip_group_check=True)
                nc.tensor.matmul(slot[0:64, 512:1024], w_ones[:, :64], w_ab[:, :],
                                 start=False, stop=True, skip_group_check=True)

            def ea_block(j, j2):
                """Two pair groups' e/a computation with one Sigmoid and one
                Tanh table load."""
                s1 = psp.tile([KF, 1024], F32, tag="ring")
                ea_mms(j, s1)
                s2 = psp.tile([KF, 1024], F32, tag="ring")
                ea_mms(j2, s2)
                # -tanh(x) = 1 - 2*sigmoid(2x): stays on the Sigmoid table
                for j_, s_ in ((j, s1), (j2, s2)):
                    nc.scalar.activation(eP[0:64, 512 * j_:512 * (j_ + 1)],
                                         s_[0:64, 0:512], ACT.Sigmoid)
                    nc.scalar.activation(naP[0:64, 512 * j_:512 * (j_ + 1)],
                                         s_[0:64, 512:1024], ACT.Sigmoid,
                                         scale=2.0)
                for j_ in (j, j2):
                    sl = naP[0:64, 512 * j_:512 * (j_ + 1)]
                    nc.gpsimd.tensor_scalar(sl, sl, -2.0, 1.0,
                                            ALU.mult, ALU.add)
                # group-2 rows start as copies of e1 / -a1 (fixup multiplies
                # in e2 / a-side later)
                for j_ in (j, j2):
                    cc0, cc1 = 512 * j_, 512 * (j_ + 1)
                    nc.gpsimd.dma_start(out=eP[64:96, cc0:cc1],
                                        in_=eP[0:32, cc0:cc1])
                    nc.gpsimd.dma_start(out=naP[64:96, cc0:cc1],
                                        in_=naP[0:32, cc0:cc1])

            def fixup_block(s):
                """Group-2 products + wd scatters for 8-pair block s.

                Pairs 8s..8s+7 = eP/naP cols [1024s, 1024s+1024), w_all cols
                [256s, 256s+256) (4 chunks), wd_dram pair range [8s, 8s+8).
                """
                c0, c1 = 1024 * s, 1024 * (s + 1)
                nc.gpsimd.dma_start(out=scrB[64:96, c0:c1], in_=eP[32:64, c0:c1])
                nc.vector.tensor_tensor(naP[64:96, c0:c1], naP[64:96, c0:c1],
                                        scrB[64:96, c0:c1], ALU.mult)
                nc.vector.tensor_tensor(eP[64:96, c0:c1], eP[64:96, c0:c1],
                                        scrB[64:96, c0:c1], ALU.mult)
                w0, w1 = 256 * s, 256 * (s + 1)
                for r0 in (0, 64):
                    nc.gpsimd.dma_start(out=wstB[r0:r0 + 32, w0:w1],
                                      in_=w_all[r0 + 32:r0 + 64, w0:w1])
                for r0 in (0, 64):
                    nc.vector.tensor_tensor(wprodB[r0:r0 + 32, w0:w1],
                                            w_all[r0:r0 + 32, w0:w1],
                                            wstB[r0:r0 + 32, w0:w1], ALU.mult)
                # diagonal scatters into wd_dram for chunks 4s..4s+3
                g0, gn = 4 * s, 4
                for parity in range(2):
                    pr = 64 * parity
                    for gi, src_t, srow in (
                        (0, w_all, pr), (1, w_all, pr + 32), (2, wprodB, pr)):
                        nc.gpsimd.dma_start(
                            out=bass.AP(wd_dram,
                                        (2 * g0 + parity) * PB + gi * 65536,
                                        [[COLS + C, NR], [2 * PB, gn], [1, C]]),
                            in_=src_t[srow:srow + 32, g0 * C:(g0 + gn) * C]
                                .rearrange("p (g c) -> p g c", c=C),
                        )

            # ---------- hoisted readout prep + counts (independent of m) ----
            def hoisted_prep():
                H = COLS // 2
                qv16 = sbp.tile([BL, KD], F16, tag="qv16")
                nc.gpsimd.indirect_dma_start(
                    out=qv16[:, :], out_offset=None,
                    in_=q_emb[:, :],
                    in_offset=bass.IndirectOffsetOnAxis(ap=idx_t_sb[:, 0:1],
                                                        axis=0),
                )
                qvT_ps = psp.tile([KD, BL], F16, tag="ring")
                nc.tensor.transpose(qvT_ps[:, :], qv16[:, :], w_id[:BL, :BL])
                nc.vector.tensor_copy(qvT_s[:, :], qvT_ps[:, :])

                # wt = softmax(qv @ key_W.T)
                zt = psp.tile([BL, C], F32, tag="ring")
                nc.tensor.matmul(zt[:, :], qvT_s[:, :], w_kWt[:, :], start=True,
                                 stop=True)
                wext = sbp.tile([BL, C], F32, tag="wext")
                smt = sbp.tile([BL, 1], F32, tag="smt")
                nc.scalar.activation(wext[:, :], zt[:, :], ACT.Exp,
                                     accum_out=smt[:, :])
                rct = sbp.tile([BL, 1], F32, tag="rct")
                nc.vector.reciprocal(rct[:, :], smt[:, :])
                wt16 = sbp.tile([BL, C], F16, tag="wt16")
                nc.vector.tensor_scalar_mul(wt16[:, :], wext[:, :], rct[:, :])
                # flatten to (1, 2048): [right blocks | wrong blocks], both = wt
                wtf = sbp.tile([1, COLS], F16, tag="wtf")
                nc.gpsimd.dma_start(out=wtf[0:1, 0:BL * C], in_=wt16[:, :])
                nc.gpsimd.dma_start(out=wtf[0:1, BL * C:COLS], in_=wt16[:, :])
                # broadcast over partitions via K=1 matmuls, one bank at a time
                for k in range(4):
                    wb_ps = psp.tile([128, 512], F32, tag="ring")
                    nc.tensor.matmul(wb_ps[:, :], w_ones[:, :],
                                     wtf[:, 512 * k:512 * (k + 1)],
                                     start=True, stop=True)
                    nc.scalar.activation(wb_s[:, 512 * k:512 * (k + 1)],
                                         wb_ps[:, :], ACT.Copy, bias=0.0)

                # global success/failure counts (use FULL inputs, same all cores)
                for ci, full in enumerate([right_full, wrong_full]):
                    fin = sbp.tile([B, S], I32, tag="fin")
                    nc.sync.dma_start(out=fin[:, :], in_=full[:, :])
                    ff = sbp.tile([B, S], F32, tag="ff")
                    nc.vector.tensor_copy(ff[:, :], fin[:, :])
                    fc = sbp.tile([B, S], F32, tag="fc")
                    nc.vector.tensor_scalar(fc[:, :], ff[:, :], 1.0, None,
                                            ALU.min)
                    cs = sbp.tile([B, 1], F32, tag="cs")
                    nc.vector.tensor_reduce(cs[:, :], fc[:, :], AX.X, ALU.add)
                    cnt_ps = psp.tile([1, 1], F32, tag="ring")
                    nc.tensor.matmul(cnt_ps[:, :], cs[:, :], w_ones_c32[:, :],
                                     start=True, stop=True)
                    nc.scalar.activation(sigs_s[:, ci:ci + 1], cnt_ps[:, :],
                                         ACT.Sigmoid)

            # ---------- emission schedule ----------
            # Quarter 0 pre-work + PF E-phases up front; quarters 1..3 are
            # split into items emitted one per pair so their work hides in
            # the loop's engine slack.
            wd_tiles = {}

            def emit_Ephase(p):
                """wd load + E-matmuls + fp16 evacuation for pair p."""
                wd4 = wdp.tile([KF, COLS], F16, tag="wd4")
                nc.sync.dma_start(
                    out=wd4[:, :],
                    in_=bass.AP(wd_dram, p * PB, [[2048, KF], [1, 2048]]),
                )
                wd_tiles[p] = wd4
                eL = eP[:, p * 128:(p + 1) * 128]
                slot = e16s[p % RING]
                for half in range(2):
                    Et = psp.tile([128, 1024], F32, tag="ring")
                    for k in range(2):
                        c0 = 1024 * half + 512 * k
                        nc.tensor.matmul(Et[:, 512 * k:512 * (k + 1)],
                                         eL, wd4[:, c0:c0 + 512],
                                         start=True, stop=True)
                    nc.scalar.activation(
                        slot[:, 1024 * half:1024 * half + 1024],
                        Et[:, :], ACT.Copy, bias=0.0)

            def emit_consume(p):
                """STT in-place on m_ps + A-matmul accumulation for pair p.

                The first 512 columns get their own STT so A0 can start while
                the wide STT covers the rest; all A-matmuls precede the next
                E-prefetch in the PE queue.
                """
                wd4 = wd_tiles.pop(p)
                aL = naP[:, p * 128:(p + 1) * 128]
                slot = e16s[p % RING]
                for half in range(2):
                    mb = mbs[half]
                    hc = 1024 * half
                    nc.vector.scalar_tensor_tensor(
                        mb[:, :], slot[:, hc:hc + 1024], 1.0,
                        mb[:, :], ALU.add, ALU.mult)
                    for k in range(2):
                        nc.tensor.matmul(mb[:, 512 * k:512 * (k + 1)],
                                         aL, wd4[:, hc + 512 * k:hc + 512 * (k + 1)],
                                         start=False, stop=True,
                                         skip_group_check=True)
                for _ in range(NDUM):
                    nc.tensor.ldweights(w_id[:, :])

            for g in range(4):
                gather_chunk(g)
            ea_block(0, 1)
            soft4(0)
            fixup_block(0)
            for p in range(PF):
                emit_Ephase(p)

            # quarter-0 residual + quarters 1..3, emitted between pairs
            work = [[(lambda g=g: gather_chunk(g)) for g in (4, 5)],
                    [(lambda g=g: gather_chunk(g)) for g in (6, 7)],
                    [lambda: ea_block(2, 3), lambda: soft4(4)],
                    [lambda: fixup_block(1), hoisted_prep]]
            sched = {p: work[p] for p in range(4)}
            for q in range(1, 4):
                qi = [(lambda g=g: gather_chunk(g))
                      for g in range(8 * q, 8 * q + 4)]
                qi.append(lambda q=q: ea_block(4 * q, 4 * q + 1))
                qi.append(lambda q=q: soft4(8 * q))
                qi.append(lambda q=q: fixup_block(2 * q))
                qi.extend((lambda g=g: gather_chunk(g))
                          for g in range(8 * q + 4, 8 * q + 8))
                qi.append(lambda q=q: ea_block(4 * q + 2, 4 * q + 3))
                qi.append(lambda q=q: soft4(8 * q + 4))
                qi.append(lambda q=q: fixup_block(2 * q + 1))
                for i, it in enumerate(qi):
                    sched.setdefault(16 * (q - 1) + 2 + i, []).append(it)

            for p in range(NP):
                for it in sched.get(p, []):
                    it()
                emit_consume(p)
                if p + PF < NP:
                    emit_Ephase(p + PF)

            # ---------- readout + head ----------
            with tc.tile_pool(name="sbr", bufs=1) as sbr:
                u2 = sbr.tile([128, COLS], F16, tag="u2")
                for h in range(2):
                    nc.vector.tensor_tensor(u2[:, 1024 * h:1024 * (h + 1)],
                                            mbs[h][:, :],
                                            wb_s[:, 1024 * h:1024 * (h + 1)],
                                            ALU.mult)
                rr = sbr.tile([VD, NR], F32, tag="rr")
                nc.vector.tensor_reduce(
                    rr[:, :], u2[:].rearrange("p (r c) -> p r c", c=C), AX.X,
                    ALU.add)
                rr16 = sbr.tile([VD, NR], F16, tag="rr16")
                nc.vector.tensor_copy(rr16[:, :], rr[:, :])

                # r_sum / w_sum: (SD, BL)
                rs_ps = psp.tile([SD, BL], F32, tag="ring")
                nc.tensor.matmul(rs_ps[:, :], w_rs0[:, :], rr16[:, 0:BL],
                                 start=True, stop=False)
                nc.tensor.matmul(rs_ps[:, :], w_rs1[:, :], qvT_s[:, :],
                                 start=False, stop=True)
                rsum = sbr.tile([SD, BL], F16, tag="rsum")
                nc.scalar.activation(rsum[:, :], rs_ps[:, :], ACT.Tanh,
                                     bias=w_rsb[:, :])
                ws_ps = psp.tile([SD, BL], F32, tag="ring")
                nc.tensor.matmul(ws_ps[:, :], w_ws0[:, :], rr16[:, BL:NR],
                                 start=True, stop=False)
                nc.tensor.matmul(ws_ps[:, :], w_ws1[:, :], qvT_s[:, :],
                                 start=False, stop=True)
                wsum = sbr.tile([SD, BL], F16, tag="wsum")
                nc.scalar.activation(wsum[:, :], ws_ps[:, :], ACT.Tanh,
                                     bias=w_wsb[:, :])

                # success/failure/difficulty levels: (1, BL)
                lv_ps = psp.tile([1, BL], F32, tag="ring")
                succ = sbr.tile([1, BL], F32, tag="succ")
                nc.tensor.matmul(lv_ps[:, :], w_succ[:, :], rsum[:, :],
                                 start=True, stop=True)
                nc.scalar.activation(succ[:, :], lv_ps[:, :], ACT.Tanh,
                                     bias=w_sb_b[:, 0:1])
                lv_ps2 = psp.tile([1, BL], F32, tag="ring")
                fail = sbr.tile([1, BL], F32, tag="fail")
                nc.tensor.matmul(lv_ps2[:, :], w_fail[:, :], wsum[:, :],
                                 start=True, stop=True)
                nc.scalar.activation(fail[:, :], lv_ps2[:, :], ACT.Tanh,
                                     bias=w_sb_b[:, 1:2])
                lv_ps3 = psp.tile([1, BL], F32, tag="ring")
                diff = sbr.tile([1, BL], F32, tag="diff")
                nc.tensor.matmul(lv_ps3[:, :], w_diff[:, :], qvT_s[:, :],
                                 start=True, stop=True)
                nc.scalar.activation(diff[:, :], lv_ps3[:, :], ACT.Tanh,
                                     bias=w_sb_b[:, 2:3])

                # out = succ*sig(sc) + fail*sig(fc) - 2*diff
                t1 = sbr.tile([1, BL], F32, tag="t1")
                nc.vector.tensor_scalar_mul(t1[:, :], succ[:, :], sigs_s[:, 0:1])
                t2 = sbr.tile([1, BL], F32, tag="t2")
                nc.vector.tensor_scalar_mul(t2[:, :], fail[:, :], sigs_s[:, 1:2])
                t3 = sbr.tile([1, BL], F32, tag="t3")
                nc.vector.tensor_scalar_mul(t3[:, :], diff[:, :], -2.0)
                o1 = sbr.tile([1, BL], F32, tag="o1")
                nc.vector.tensor_tensor(o1[:, :], t1[:, :], t2[:, :], ALU.add)
                o2 = sbr.tile([1, BL], F32, tag="o2")
                nc.vector.tensor_tensor(o2[:, :], o1[:, :], t3[:, :], ALU.add)
                nc.sync.dma_start(out=out_d[:, :], in_=o2[:, :])

                if DEBUG:
                    mdbg = sbr.tile([128, COLS], F16, tag="mdbg")
                    for h in range(2):
                        nc.vector.tensor_copy(mdbg[:, 1024 * h:1024 * (h + 1)],
                                              mbs[h][:, :])
                    nc.sync.dma_start(out=dbg_m[:, :], in_=mdbg[:, :])
                    nc.sync.dma_start(out=dbg_rr[:, :], in_=rr[:, :])

    nc.compile()
    return nc


_WD_ZERO = np.zeros(NP * PB, dtype=np.float16)

_PROGRAM = None


def _get_program():
    global _PROGRAM
    if _PROGRAM is None:
        _PROGRAM = _build_program()
    return _PROGRAM


def _host_inputs(inputs):
    """Build the per-core in_maps from the full problem inputs."""
    f16 = np.float16
    f32 = np.float32
    ri = np.asarray(inputs["right_input"]).astype(np.int64)
    wi = np.asarray(inputs["wrong_input"]).astype(np.int64)
    tg = np.asarray(inputs["target_id"]).astype(np.int64)
    q_emb = np.asarray(inputs["q_emb"], dtype=f32)
    i_emb = np.asarray(inputs["i_emb"], dtype=f32)

    def W(name):
        return np.asarray(inputs[name], dtype=f32)

    common = {
        "i_emb": i_emb.astype(f16),
        "q_emb": q_emb.astype(f16),
        "wd_dram": _WD_ZERO,
        "erase_Wt": np.ascontiguousarray(W("erase_W").T).astype(f16),
        "add_Wt": np.ascontiguousarray(W("add_W").T).astype(f16),
        "key_Wt": np.ascontiguousarray(W("key_W").T).astype(f16),
        "erase_b_row": np.tile(W("erase_b").reshape(1, -1), (1, 4)).astype(f16),
        "add_b_row": np.tile(W("add_b").reshape(1, -1), (1, 4)).astype(f16),
        "rsum_Wt0": np.ascontiguousarray(W("rsum_W")[:, :VD].T).astype(f16),
        "rsum_Wt1": np.ascontiguousarray(W("rsum_W")[:, VD:].T).astype(f16),
        "wsum_Wt0": np.ascontiguousarray(W("wsum_W")[:, :VD].T).astype(f16),
        "wsum_Wt1": np.ascontiguousarray(W("wsum_W")[:, VD:].T).astype(f16),
        "rsum_b_col": W("rsum_b").reshape(-1, 1).astype(f32),
        "wsum_b_col": W("wsum_b").reshape(-1, 1).astype(f32),
        "succ_Wt": np.ascontiguousarray(W("succ_W").T).astype(f16),
        "fail_Wt": np.ascontiguousarray(W("fail_W").T).astype(f16),
        "diff_Wt": np.ascontiguousarray(W("diff_W").T).astype(f16),
        "succ_b": W("succ_b").reshape(1, 1).astype(f32),
        "fail_b": W("fail_b").reshape(1, 1).astype(f32),
        "diff_b": W("diff_b").reshape(1, 1).astype(f32),
        "rmem0": W("right_mem_init").astype(f16),
        "wmem0": W("wrong_mem_init").astype(f16),
        "ones_row": np.ones((1, 128), dtype=f16),
        "ones_col32": np.ones((128, 1), dtype=f32),
        "id128": np.eye(128, dtype=f16),
        "right_full": ri.astype(np.int32),
        "wrong_full": wi.astype(np.int32),
    }

    in_maps = []
    for core in range(NCORE):
        rows = slice(core * BL, (core + 1) * BL)
        # inter ids per (t, r): r<BL -> right, else wrong
        inter = np.empty((S, NR), dtype=np.int64)
        inter[:, :BL] = ri[rows].T
        inter[:, BL:] = wi[rows].T
        qid = inter - Q * (inter > Q)
        # both tables: t-major 128-row chunks (4 steps x 32 rows per chunk)
        idx_ip = inter.reshape(-1).reshape(NGQ, 128).T.astype(np.int32)
        flat_q = qid.reshape(-1)
        idx_q = flat_q.reshape(NGQ, 128).T.astype(np.int32)
        idx_t = tg[rows].reshape(BL, 1).astype(np.int32)
        in_maps.append({**common, "idx_i": np.ascontiguousarray(idx_ip),
                        "idx_q": np.ascontiguousarray(idx_q),
                        "idx_t": idx_t})
    return in_maps


def run_spmd(inputs, trace=False):
    nc = _get_program()
    in_maps = _host_inputs(inputs)
    res = run_bass_kernel_spmd(nc, in_maps, core_ids=list(range(NCORE)),
                               trace=trace)
    out = np.concatenate([res.results[i]["out"] for i in range(NCORE)], axis=0)
    return out.astype(np.float32), res


def kernel(**inputs):
    out, _ = run_spmd(inputs, trace=False)
    return out


# revision 20
# speedup vs baseline: 1.1307x; 1.1307x over previous
"""DiKT (DKVMN-style knowledge tracing) Trainium2 kernel.

Self-contained: builds a Bass/Tile program, shards batch over 8 NeuronCores
(pure data parallel, 16 batch rows per core), runs via run_bass_kernel_spmd.

Algorithm per core (B_loc=16, V=128, C=64, S=128 steps):
  Both value memories (right/wrong) live as ONE SBUF tensor m[v=128, col=2048]
  with col = r*64 + c, r = mem*16 + b.  Per step:
     m' = m * (1 - e x w) + a x w

Two consecutive steps are FUSED into one update (64 pairs):
     Shat = S1*S2 = 1 - e1 x w1 - e2 x w2 + (e1e2) x (w1w2)
     Ahat = S2*A1 + A2 = a1 x w1 - (a1e2) x (w1w2) + a2 x w2
Both are rank-3 sums of outer products, built by ONE K=96 TensorE matmul
against a 3-group block-diagonal rhs (per group g, row r: the (r,c) diagonal
block carries [-w1 | -w2 | +w1w2]).  lhsT groups: [e1 | e2 | e1e2] for Shat,
[-a1 | -a2 | -a1e2] for Ahat (signs make every product come out right).

Consumption per pair uses PSUM-resident accumulation (validated on HW):
  E-matmul writes Shat-1 into a PSUM bank; a DVE scalar_tensor_tensor runs
  IN-PLACE on the bank: bank = (bank + 1) * m; the A-matmul then
  accumulates Ahat onto the same bank (start=False, skip_group_check); one
  1024-col ACT copy moves the finished columns back to fp16 m.
  This costs 1 DVE op + 1 copy per 512-col chunk instead of the previous
  2 elementwise ops + 2 ACT evacuations.

The TensorE p-state ramp (2.4GHz only after ~3us of gapless execution, else
1.2GHz) is held at full clock by filler matmuls into a scratch bank whenever
the real per-pair matmul work would leave gaps.

The pre-phase (embedding gathers, transposes, e/a/w computation, and the
block-diagonal wd scatter to DRAM) is emitted interleaved with the pair loop
in per-4-pair granules so it executes in the other engines' slack instead of
serializing in front of the loop.
"""

import numpy as np

import concourse.mybir as mybir
from concourse import bass, bacc, tile
from concourse.bass_utils import run_bass_kernel_spmd

F16 = mybir.dt.float16
F32 = mybir.dt.float32
I32 = mybir.dt.int32
ALU = mybir.AluOpType
ACT = mybir.ActivationFunctionType
AX = mybir.AxisListType

# model dims
KD = 128      # KEY_DIM
VD = 128      # VALUE_DIM
SD = 128      # SUMMARY_DIM
Q = 10000     # QUESTION_NUM
C = 64        # CONCEPT_NUM
B = 128       # full batch
S = 128       # seq len
NCORE = 8
BL = B // NCORE          # 16 batch rows per core
NR = 2 * BL              # 32 rows per step (right+wrong)
COLS = NR * C            # 2048 memory columns per core
NP = S // 2              # 64 step pairs
KF = 3 * NR              # 96 contraction rows per fused matmul
PB = KF * COLS           # 196608 elements per pair block in wd_dram
NGQ = (S * NR) // 128    # 32 q-side gather chunks of 128 rows
NPG = NP // 4            # 16 e/a matmul groups of 4 pairs
NDUM = 1                 # filler ldweights per pair for the PE p-state hold
RING = 6                 # e16 staging depth in pairs
PF = 4                   # E-matmul prefetch distance in pairs

import os
DEBUG = bool(os.environ.get("KDEBUG"))


def _build_program():
    nc = bacc.Bacc(trn_type="TRN2", target_bir_lowering=False, num_devices=NCORE,
                   num_swdge_queues=4)

    # ---- DRAM inputs ----
    i_emb = nc.dram_tensor("i_emb", [2 * Q + 1, VD], F16, kind="ExternalInput")
    q_emb = nc.dram_tensor("q_emb", [Q + 1, KD], F16, kind="ExternalInput")
    idx_i = nc.dram_tensor("idx_i", [128, NGQ], I32, kind="ExternalInput")
    idx_q = nc.dram_tensor("idx_q", [128, NGQ], I32, kind="ExternalInput")
    idx_t = nc.dram_tensor("idx_t", [BL, 1], I32, kind="ExternalInput")

    erase_Wt = nc.dram_tensor("erase_Wt", [VD, VD], F16, kind="ExternalInput")
    add_Wt = nc.dram_tensor("add_Wt", [VD, VD], F16, kind="ExternalInput")
    key_Wt = nc.dram_tensor("key_Wt", [KD, C], F16, kind="ExternalInput")
    erase_b_row = nc.dram_tensor("erase_b_row", [1, 4 * VD], F16, kind="ExternalInput")
    add_b_row = nc.dram_tensor("add_b_row", [1, 4 * VD], F16, kind="ExternalInput")
    rsum_Wt0 = nc.dram_tensor("rsum_Wt0", [VD, SD], F16, kind="ExternalInput")
    rsum_Wt1 = nc.dram_tensor("rsum_Wt1", [KD, SD], F16, kind="ExternalInput")
    wsum_Wt0 = nc.dram_tensor("wsum_Wt0", [VD, SD], F16, kind="ExternalInput")
    wsum_Wt1 = nc.dram_tensor("wsum_Wt1", [KD, SD], F16, kind="ExternalInput")
    rsum_b_col = nc.dram_tensor("rsum_b_col", [SD, 1], F32, kind="ExternalInput")
    wsum_b_col = nc.dram_tensor("wsum_b_col", [SD, 1], F32, kind="ExternalInput")
    succ_Wt = nc.dram_tensor("succ_Wt", [SD, 1], F16, kind="ExternalInput")
    fail_Wt = nc.dram_tensor("fail_Wt", [SD, 1], F16, kind="ExternalInput")
    diff_Wt = nc.dram_tensor("diff_Wt", [KD, 1], F16, kind="ExternalInput")
    succ_b = nc.dram_tensor("succ_b", [1, 1], F32, kind="ExternalInput")
    fail_b = nc.dram_tensor("fail_b", [1, 1], F32, kind="ExternalInput")
    diff_b = nc.dram_tensor("diff_b", [1, 1], F32, kind="ExternalInput")
    rmem0 = nc.dram_tensor("rmem0", [VD, C], F16, kind="ExternalInput")
    wmem0 = nc.dram_tensor("wmem0", [VD, C], F16, kind="ExternalInput")
    ones_row = nc.dram_tensor("ones_row", [1, 128], F16, kind="ExternalInput")
    ones_col32 = nc.dram_tensor("ones_col32", [128, 1], F32, kind="ExternalInput")
    id128 = nc.dram_tensor("id128", [128, 128], F16, kind="ExternalInput")
    right_full = nc.dram_tensor("right_full", [B, S], I32, kind="ExternalInput")
    wrong_full = nc.dram_tensor("wrong_full", [B, S], I32, kind="ExternalInput")

    out_d = nc.dram_tensor("out", [BL, 1], F32, kind="ExternalOutput")
    if DEBUG:
        dbg_m = nc.dram_tensor("dbg_m", [VD, COLS], F16, kind="ExternalOutput")
        dbg_rr = nc.dram_tensor("dbg_rr", [VD, NR], F32, kind="ExternalOutput")

    # fused block-diagonal rhs for every pair, flat fp16:
    # pair p, group g (0..2), row r (0..31): diag block at
    #   p*PB + g*65536 + r*2112, 64 wide
    wd_dram = nc.dram_tensor("wd_dram", [NP * PB], F16, kind="ExternalInput")

    # ---- persistent SBUF ----
    sb = lambda name, shape, dt: nc.alloc_sbuf_tensor(name, shape, dt)
    m0b = sb("m0b", [VD, COLS], F16)          # broadcast initial memory
    # staged Shat-1 per pair: one tensor per ring slot (dep isolation)
    e16s = [sb(f"e16_{s}", [128, COLS], F16) for s in range(RING)]
    vecT = sb("vecT", [128, NP * 64], F16)   # i_emb rows [t1|t2] per pair, transposed
    qT = sb("qT", [128, NGQ * 128], F16)
    eP = sb("eP", [KF, NP * 128], F16)       # [e1 | e2 | e1e2] per pair
    naP = sb("naP", [KF, NP * 128], F16)     # [-a1 | -a2 | -a1e2] per pair
    scrB = sb("scrB", [KF, NP * 128], F16)   # e2 staging at group-2 partitions
    w_all = sb("w_all", [128, NGQ * C], F16)  # -w per step (4 steps/chunk)
    wstB = sb("wstB", [KF, NGQ * C], F16)    # -w2 at w1 partitions
    wprodB = sb("wprodB", [KF, NGQ * C], F16)  # w1*w2
    w_eWt = sb("w_eWt", [VD, VD], F16)
    w_aWt = sb("w_aWt", [VD, VD], F16)
    w_kWt = sb("w_kWt", [KD, C], F16)
    w_eb = sb("w_eb", [1, 4 * VD], F16)
    w_ab = sb("w_ab", [1, 4 * VD], F16)
    w_ones = sb("w_ones", [1, 128], F16)
    w_ones_c32 = sb("w_ones_c32", [128, 1], F32)
    w_id = sb("w_id", [128, 128], F16)
    idx_i_sb = sb("idx_i_sb", [128, NGQ], I32)
    idx_q_sb = sb("idx_q_sb", [128, NGQ], I32)
    idx_t_sb = sb("idx_t_sb", [BL, 1], I32)
    w_rs0 = sb("w_rs0", [VD, SD], F16)
    w_rs1 = sb("w_rs1", [KD, SD], F16)
    w_ws0 = sb("w_ws0", [VD, SD], F16)
    w_ws1 = sb("w_ws1", [KD, SD], F16)
    w_rsb = sb("w_rsb", [SD, 1], F32)
    w_wsb = sb("w_wsb", [SD, 1], F32)
    w_succ = sb("w_succ", [SD, 1], F16)
    w_fail = sb("w_fail", [SD, 1], F16)
    w_diff = sb("w_diff", [KD, 1], F16)
    w_sb_b = sb("w_sb_b", [1, 3], F32)  # succ_b, fail_b, diff_b columns 0..2
    wb_s = sb("wb_s", [128, COLS], F16)   # target corr weights, broadcast
    qvT_s = sb("qvT_s", [KD, BL], F16)    # target question emb, transposed
    sigs_s = sb("sigs_s", [1, 2], F32)    # sigmoid(success/failure counts)

    # ---- persistent PSUM: the memory itself lives here (4 banks).
    # Two independent half tensors so the dependency tracker (whole-tensor
    # granularity for persistent tensors) does not serialize the halves.
    mbs = [nc.alloc_psum_tensor(f"mb{h}", [128, 1024], F32) for h in range(2)]

    with tile.TileContext(nc) as tc:
        with tc.tile_pool(name="sbp", bufs=3) as sbp, \
             tc.tile_pool(name="psp", bufs=2, space="PSUM") as psp, \
             tc.tile_pool(name="wdp", bufs=PF + 2) as wdp:

            # ---------- constant loads ----------
            # gather/transpose-critical consts first on the sync queue; the
            # rest ride the gpsimd queue so the bootstrap gathers start
            # immediately.
            for dst, src in [
                (idx_i_sb, idx_i), (idx_q_sb, idx_q), (idx_t_sb, idx_t),
                (w_id, id128), (w_eWt, erase_Wt), (w_aWt, add_Wt),
                (w_kWt, key_Wt), (w_eb, erase_b_row), (w_ab, add_b_row),
                (w_ones, ones_row),
            ]:
                nc.sync.dma_start(out=dst[:, :], in_=src[:, :])
            for dst, src in [
                (w_ones_c32, ones_col32),
                (w_rs0, rsum_Wt0), (w_rs1, rsum_Wt1),
                (w_ws0, wsum_Wt0), (w_ws1, wsum_Wt1),
                (w_rsb, rsum_b_col), (w_wsb, wsum_b_col),
                (w_succ, succ_Wt), (w_fail, fail_Wt), (w_diff, diff_Wt),
            ]:
                nc.gpsimd.dma_start(out=dst[:, :], in_=src[:, :])
            nc.gpsimd.dma_start(out=w_sb_b[:, 0:1], in_=succ_b[:, :])
            nc.gpsimd.dma_start(out=w_sb_b[:, 1:2], in_=fail_b[:, :])
            nc.gpsimd.dma_start(out=w_sb_b[:, 2:3], in_=diff_b[:, :])

            # warm the PE clock while the boot DMAs run
            for _ in range(36):
                nc.tensor.ldweights(w_id[:, :])

            # init m: broadcast mem inits over the 16 batch blocks
            rmem_t = sbp.tile([VD, C], F16, tag="memi")
            nc.sync.dma_start(out=rmem_t[:, :], in_=rmem0[:, :])
            wmem_t = sbp.tile([VD, C], F16, tag="memi2")
            nc.sync.dma_start(out=wmem_t[:, :], in_=wmem0[:, :])
            for r in range(NR):
                srct = rmem_t if r < BL else wmem_t
                nc.vector.tensor_copy(m0b[:, r * C:(r + 1) * C], srct[:, :])
            for k in range(4):
                nc.tensor.matmul(mbs[k // 2][:, 512 * (k % 2):512 * (k % 2 + 1)],
                                 w_id[:, :], m0b[:, 512 * k:512 * (k + 1)],
                                 start=True, stop=True)

            # ---------- pre-phase emission helpers ----------
            def gather_i(g):
                """i-side gather + PE transpose for 128-row chunk g."""
                lo = g * 128
                gi16 = sbp.tile([128, VD], F16, tag="gi16")
                nc.gpsimd.indirect_dma_start(
                    out=gi16[:, :], out_offset=None,
                    in_=i_emb[:, :],
                    in_offset=bass.IndirectOffsetOnAxis(
                        ap=idx_i_sb[:, g:g + 1], axis=0),
                )
                tps = psp.tile([128, 128], F16, tag="ring")
                nc.tensor.transpose(tps[:, :], gi16[:, :], w_id[:, :])
                nc.vector.tensor_copy(vecT[:, lo:lo + 128], tps[:, :])

            def gather_q(g):
                """q-side gather + PE transpose for 128-row chunk g."""
                lo = g * 128
                gq16 = sbp.tile([128, KD], F16, tag="gq16")
                nc.gpsimd.indirect_dma_start(
                    out=gq16[:, :], out_offset=None,
                    in_=q_emb[:, :],
                    in_offset=bass.IndirectOffsetOnAxis(
                        ap=idx_q_sb[:, g:g + 1], axis=0),
                )
                tps2 = psp.tile([128, 128], F16, tag="ring")
                nc.tensor.transpose(tps2[:, :], gq16[:, :], w_id[:, :])
                nc.vector.tensor_copy(qT[:, lo:lo + 128], tps2[:, :])

            def gather_chunk(g):
                gather_i(g)
                gather_q(g)

            def soft4(g0):
                """w = -softmax(qv @ key_W.T) for chunks g0..g0+3, one Exp
                table load for the batch."""
                zq = psp.tile([128, 256], F32, tag="ring")
                for i in range(4):
                    g = g0 + i
                    nc.tensor.matmul(zq[:, 64 * i:64 * (i + 1)],
                                     qT[:, g * 128:(g + 1) * 128], w_kWt[:, :],
                                     start=True, stop=True)
                wexq = sbp.tile([128, 256], F32, tag="wexq")
                nc.scalar.activation(wexq[:, :], zq[:, :], ACT.Exp)
                smq = sbp.tile([128, 4], F32, tag="smq")
                nc.vector.tensor_reduce(
                    smq[:, :], wexq[:].rearrange("p (g c) -> p g c", c=C),
                    AX.X, ALU.add)
                rcq = sbp.tile([128, 4], F32, tag="rcq")
                nc.vector.reciprocal(rcq[:, :], smq[:, :])
                for i in range(4):
                    g = g0 + i
                    nc.gpsimd.tensor_scalar(w_all[:, g * C:(g + 1) * C],
                                            wexq[:, 64 * i:64 * (i + 1)],
                                            rcq[:, i:i + 1], -1.0,
                                            ALU.mult, ALU.mult)

            def ea_mms(j, slot):
                """e (cols 0:512) and a (cols 512:1024) matmuls for pair
                group j into one ring slot."""
                for k in range(4):
                    p = 4 * j + k
                    vl = vecT[:, 64 * p:64 * (p + 1)]
                    nc.tensor.matmul(slot[0:64, 128 * k:128 * (k + 1)],
                                     vl, w_eWt[:, :], start=True, stop=True)
                    nc.tensor.matmul(slot[0:64, 512 + 128 * k:512 + 128 * (k + 1)],
                                     vl, w_aWt[:, :], start=True, stop=True)
                nc.tensor.matmul(slot[0:64, 0:512], w_ones[:, :64], w_eb[:, :],
                                 start=False, stop=True, skip_group_check=True)
                nc.tensor.matmul(slot[0:64, 512:1024], w_ones[:, :64], w_ab[:, :],
                                 start=False, stop=True, skip_group_check=True)

            def ea_block(j, j2):
                """Two pair groups' e/a computation with one Sigmoid and one
                Tanh table load."""
                s1 = psp.tile([KF, 1024], F32, tag="ring")
                ea_mms(j, s1)
                s2 = psp.tile([KF, 1024], F32, tag="ring")
                ea_mms(j2, s2)
                # -tanh(x) = 1 - 2*sigmoid(2x): stays on the Sigmoid table
                for j_, s_ in ((j, s1), (j2, s2)):
                    nc.scalar.activation(eP[0:64, 512 * j_:512 * (j_ + 1)],
                                         s_[0:64, 0:512], ACT.Sigmoid)
                    nc.scalar.activation(naP[0:64, 512 * j_:512 * (j_ + 1)],
                                         s_[0:64, 512:1024], ACT.Sigmoid,
                                         scale=2.0)
                for j_ in (j, j2):
                    sl = naP[0:64, 512 * j_:512 * (j_ + 1)]
                    nc.gpsimd.tensor_scalar(sl, sl, -2.0, 1.0,
                                            ALU.mult, ALU.add)
                # group-2 rows start as copies of e1 / -a1 (fixup multiplies
                # in e2 / a-side later)
                for j_ in (j, j2):
                    cc0, cc1 = 512 * j_, 512 * (j_ + 1)
                    nc.gpsimd.dma_start(out=eP[64:96, cc0:cc1],
                                        in_=eP[0:32, cc0:cc1])
                    nc.gpsimd.dma_start(out=naP[64:96, cc0:cc1],
                                        in_=naP[0:32, cc0:cc1])

            def fixup_block(s):
                """Group-2 products + wd scatters for 8-pair block s.

                Pairs 8s..8s+7 = eP/naP cols [1024s, 1024s+1024), w_all cols
                [256s, 256s+256) (4 chunks), wd_dram pair range [8s, 8s+8).
                """
                c0, c1 = 1024 * s, 1024 * (s + 1)
                nc.gpsimd.dma_start(out=scrB[64:96, c0:c1], in_=eP[32:64, c0:c1])
                nc.vector.tensor_tensor(naP[64:96, c0:c1], naP[64:96, c0:c1],
                                        scrB[64:96, c0:c1], ALU.mult)
                nc.vector.tensor_tensor(eP[64:96, c0:c1], eP[64:96, c0:c1],
                                        scrB[64:96, c0:c1], ALU.mult)
                w0, w1 = 256 * s, 256 * (s + 1)
                for r0 in (0, 64):
                    nc.gpsimd.dma_start(out=wstB[r0:r0 + 32, w0:w1],
                                      in_=w_all[r0 + 32:r0 + 64, w0:w1])
                for r0 in (0, 64):
                    nc.vector.tensor_tensor(wprodB[r0:r0 + 32, w0:w1],
                                            w_all[r0:r0 + 32, w0:w1],
                                            wstB[r0:r0 + 32, w0:w1], ALU.mult)
                # diagonal scatters into wd_dram for chunks 4s..4s+3
                g0, gn = 4 * s, 4
                for parity in range(2):
                    pr = 64 * parity
                    for gi, src_t, srow in (
                        (0, w_all, pr), (1, w_all, pr + 32), (2, wprodB, pr)):
                        nc.gpsimd.dma_start(
                            out=bass.AP(wd_dram,
                                        (2 * g0 + parity) * PB + gi * 65536,
                                        [[COLS + C, NR], [2 * PB, gn], [1, C]]),
                            in_=src_t[srow:srow + 32, g0 * C:(g0 + gn) * C]
                                .rearrange("p (g c) -> p g c", c=C),
                        )

            # ---------- hoisted readout prep + counts (independent of m) ----
            def hoisted_prep():
                H = COLS // 2
                qv16 = sbp.tile([BL, KD], F16, tag="qv16")
                nc.gpsimd.indirect_dma_start(
                    out=qv16[:, :], out_offset=None,
                    in_=q_emb[:, :],
                    in_offset=bass.IndirectOffsetOnAxis(ap=idx_t_sb[:, 0:1],
                                                        axis=0),
                )
                qvT_ps = psp.tile([KD, BL], F16, tag="ring")
                nc.tensor.transpose(qvT_ps[:, :], qv16[:, :], w_id[:BL, :BL])
                nc.vector.tensor_copy(qvT_s[:, :], qvT_ps[:, :])

                # wt = softmax(qv @ key_W.T)
                zt = psp.tile([BL, C], F32, tag="ring")
                nc.tensor.matmul(zt[:, :], qvT_s[:, :], w_kWt[:, :], start=True,
                                 stop=True)
                wext = sbp.tile([BL, C], F32, tag="wext")
                smt = sbp.tile([BL, 1], F32, tag="smt")
                nc.scalar.activation(wext[:, :], zt[:, :], ACT.Exp,
                                     accum_out=smt[:, :])
                rct = sbp.tile([BL, 1], F32, tag="rct")
                nc.vector.reciprocal(rct[:, :], smt[:, :])
                wt16 = sbp.tile([BL, C], F16, tag="wt16")
                nc.vector.tensor_scalar_mul(wt16[:, :], wext[:, :], rct[:, :])
                # flatten to (1, 2048): [right blocks | wrong blocks], both = wt
                wtf = sbp.tile([1, COLS], F16, tag="wtf")
                nc.gpsimd.dma_start(out=wtf[0:1, 0:BL * C], in_=wt16[:, :])
                nc.gpsimd.dma_start(out=wtf[0:1, BL * C:COLS], in_=wt16[:, :])
                # broadcast over partitions via K=1 matmuls, one bank at a time
                for k in range(4):
                    wb_ps = psp.tile([128, 512], F32, tag="ring")
                    nc.tensor.matmul(wb_ps[:, :], w_ones[:, :],
                                     wtf[:, 512 * k:512 * (k + 1)],
                                     start=True, stop=True)
                    nc.scalar.activation(wb_s[:, 512 * k:512 * (k + 1)],
                                         wb_ps[:, :], ACT.Copy, bias=0.0)

                # global success/failure counts (use FULL inputs, same all cores)
                for ci, full in enumerate([right_full, wrong_full]):
                    fin = sbp.tile([B, S], I32, tag="fin")
                    nc.sync.dma_start(out=fin[:, :], in_=full[:, :])
                    ff = sbp.tile([B, S], F32, tag="ff")
                    nc.vector.tensor_copy(ff[:, :], fin[:, :])
                    fc = sbp.tile([B, S], F32, tag="fc")
                    nc.vector.tensor_scalar(fc[:, :], ff[:, :], 1.0, None,
                                            ALU.min)
                    cs = sbp.tile([B, 1], F32, tag="cs")
                    nc.vector.tensor_reduce(cs[:, :], fc[:, :], AX.X, ALU.add)
                    cnt_ps = psp.tile([1, 1], F32, tag="ring")
                    nc.tensor.matmul(cnt_ps[:, :], cs[:, :], w_ones_c32[:, :],
                                     start=True, stop=True)
                    nc.scalar.activation(sigs_s[:, ci:ci + 1], cnt_ps[:, :],
                                         ACT.Sigmoid)

            # ---------- emission schedule ----------
            # Quarter 0 pre-work + PF E-phases up front; quarters 1..3 are
            # split into items emitted one per pair so their work hides in
            # the loop's engine slack.
            wd_tiles = {}

            def emit_Ephase(p):
                """wd load + E-matmuls + fp16 evacuation for pair p."""
                wd4 = wdp.tile([KF, COLS], F16, tag="wd4")
                nc.sync.dma_start(
                    out=wd4[:, :],
                    in_=bass.AP(wd_dram, p * PB, [[2048, KF], [1, 2048]]),
                )
                wd_tiles[p] = wd4
                eL = eP[:, p * 128:(p + 1) * 128]
                slot = e16s[p % RING]
                for half in range(2):
                    Et = psp.tile([128, 1024], F32, tag="ring")
                    for k in range(2):
                        c0 = 1024 * half + 512 * k
                        nc.tensor.matmul(Et[:, 512 * k:512 * (k + 1)],
                                         eL, wd4[:, c0:c0 + 512],
                                         start=True, stop=True)
                    nc.scalar.activation(
                        slot[:, 1024 * half:1024 * half + 1024],
                        Et[:, :], ACT.Copy, bias=0.0)

            def emit_consume(p):
                """STT in-place on m_ps + A-matmul accumulation for pair p.

                The first 512 columns get their own STT so A0 can start while
                the wide STT covers the rest; all A-matmuls precede the next
                E-prefetch in the PE queue.
                """
                wd4 = wd_tiles.pop(p)
                aL = naP[:, p * 128:(p + 1) * 128]
                slot = e16s[p % RING]
                for half in range(2):
                    mb = mbs[half]
                    hc = 1024 * half
                    nc.vector.scalar_tensor_tensor(
                        mb[:, :], slot[:, hc:hc + 1024], 1.0,
                        mb[:, :], ALU.add, ALU.mult)
                    for k in range(2):
                        nc.tensor.matmul(mb[:, 512 * k:512 * (k + 1)],
                                         aL, wd4[:, hc + 512 * k:hc + 512 * (k + 1)],
                                         start=False, stop=True,
                                         skip_group_check=True)
                for _ in range(NDUM):
                    nc.tensor.ldweights(w_id[:, :])

            for g in range(4):
                gather_chunk(g)
            ea_block(0, 1)
            soft4(0)
            fixup_block(0)
            for p in range(PF):
                emit_Ephase(p)

            # quarter-0 residual + quarters 1..3, emitted between pairs
            work = [[(lambda g=g: gather_chunk(g)) for g in (4, 5)],
                    [(lambda g=g: gather_chunk(g)) for g in (6, 7)],
                    [lambda: ea_block(2, 3), lambda: soft4(4)],
                    [lambda: fixup_block(1), hoisted_prep]]
            sched = {p: work[p] for p in range(4)}
            for q in range(1, 4):
                qi = [(lambda g=g: gather_chunk(g))
                      for g in range(8 * q, 8 * q + 4)]
                qi.append(lambda q=q: ea_block(4 * q, 4 * q + 1))
                qi.append(lambda q=q: soft4(8 * q))
                qi.append(lambda q=q: fixup_block(2 * q))
                qi.extend((lambda g=g: gather_chunk(g))
                          for g in range(8 * q + 4, 8 * q + 8))
                qi.append(lambda q=q: ea_block(4 * q + 2, 4 * q + 3))
                qi.append(lambda q=q: soft4(8 * q + 4))
                qi.append(lambda q=q: fixup_block(2 * q + 1))
                for i, it in enumerate(qi):
                    sched.setdefault(16 * (q - 1) + 2 + i, []).append(it)

            for p in range(NP):
                for it in sched.get(p, []):
                    it()
                emit_consume(p)
                if p + PF < NP:
                    emit_Ephase(p + PF)

            # ---------- readout + head ----------
            with tc.tile_pool(name="sbr", bufs=1) as sbr:
                u2 = sbr.tile([128, COLS], F16, tag="u2")
                for h in range(2):
                    nc.vector.tensor_tensor(u2[:, 1024 * h:1024 * (h + 1)],
                                            mbs[h][:, :],
                                            wb_s[:, 1024 * h:1024 * (h + 1)],
                                            ALU.mult)
                rr = sbr.tile([VD, NR], F32, tag="rr")
                nc.vector.tensor_reduce(
                    rr[:, :], u2[:].rearrange("p (r c) -> p r c", c=C), AX.X,
                    ALU.add)
                rr16 = sbr.tile([VD, NR], F16, tag="rr16")
                nc.vector.tensor_copy(rr16[:, :], rr[:, :])

                # r_sum / w_sum: (SD, BL)
                rs_ps = psp.tile([SD, BL], F32, tag="ring")
                nc.tensor.matmul(rs_ps[:, :], w_rs0[:, :], rr16[:, 0:BL],
                                 start=True, stop=False)
                nc.tensor.matmul(rs_ps[:, :], w_rs1[:, :], qvT_s[:, :],
                                 start=False, stop=True)
                rsum = sbr.tile([SD, BL], F16, tag="rsum")
                nc.scalar.activation(rsum[:, :], rs_ps[:, :], ACT.Tanh,
                                     bias=w_rsb[:, :])
                ws_ps = psp.tile([SD, BL], F32, tag="ring")
                nc.tensor.matmul(ws_ps[:, :], w_ws0[:, :], rr16[:, BL:NR],
                                 start=True, stop=False)
                nc.tensor.matmul(ws_ps[:, :], w_ws1[:, :], qvT_s[:, :],
                                 start=False, stop=True)
                wsum = sbr.tile([SD, BL], F16, tag="wsum")
                nc.scalar.activation(wsum[:, :], ws_ps[:, :], ACT.Tanh,
                                     bias=w_wsb[:, :])

                # success/failure/difficulty levels: (1, BL)
                lv_ps = psp.tile([1, BL], F32, tag="ring")
                succ = sbr.tile([1, BL], F32, tag="succ")
                nc.tensor.matmul(lv_ps[:, :], w_succ[:, :], rsum[:, :],
                                 start=True, stop=True)
                nc.scalar.activation(succ[:, :], lv_ps[:, :], ACT.Tanh,
                                     bias=w_sb_b[:, 0:1])
                lv_ps2 = psp.tile([1, BL], F32, tag="ring")
                fail = sbr.tile([1, BL], F32, tag="fail")
                nc.tensor.matmul(lv_ps2[:, :], w_fail[:, :], wsum[:, :],
                                 start=True, stop=True)
                nc.scalar.activation(fail[:, :], lv_ps2[:, :], ACT.Tanh,
                                     bias=w_sb_b[:, 1:2])
                lv_ps3 = psp.tile([1, BL], F32, tag="ring")
                diff = sbr.tile([1, BL], F32, tag="diff")
                nc.tensor.matmul(lv_ps3[:, :], w_diff[:, :], qvT_s[:, :],
                                 start=True, stop=True)
                nc.scalar.activation(diff[:, :], lv_ps3[:, :], ACT.Tanh,
                                     bias=w_sb_b[:, 2:3])

                # out = succ*sig(sc) + fail*sig(fc) - 2*diff
                t1 = sbr.tile([1, BL], F32, tag="t1")
                nc.vector.tensor_scalar_mul(t1[:, :], succ[:, :], sigs_s[:, 0:1])
                t2 = sbr.tile([1, BL], F32, tag="t2")
                nc.vector.tensor_scalar_mul(t2[:, :], fail[:, :], sigs_s[:, 1:2])
                t3 = sbr.tile([1, BL], F32, tag="t3")
                nc.vector.tensor_scalar_mul(t3[:, :], diff[:, :], -2.0)
                o1 = sbr.tile([1, BL], F32, tag="o1")
                nc.vector.tensor_tensor(o1[:, :], t1[:, :], t2[:, :], ALU.add)
                o2 = sbr.tile([1, BL], F32, tag="o2")
                nc.vector.tensor_tensor(o2[:, :], o1[:, :], t3[:, :], ALU.add)
                nc.sync.dma_start(out=out_d[:, :], in_=o2[:, :])

                if DEBUG:
                    mdbg = sbr.tile([128, COLS], F16, tag="mdbg")
                    for h in range(2):
                        nc.vector.tensor_copy(mdbg[:, 1024 * h:1024 * (h + 1)],
                                              mbs[h][:, :])
                    nc.sync.dma_start(out=dbg_m[:, :], in_=mdbg[:, :])
                    nc.sync.dma_start(out=dbg_rr[:, :], in_=rr[:, :])

    nc.compile()
    return nc


_WD_ZERO = np.zeros(NP * PB, dtype=np.float16)

_PROGRAM = None


def _get_program():
    global _PROGRAM
    if _PROGRAM is None:
        _PROGRAM = _build_program()
    return _PROGRAM


def _host_inputs(inputs):
    """Build the per-core in_maps from the full problem inputs."""
    f16 = np.float16
    f32 = np.float32
    ri = np.asarray(inputs["right_input"]).astype(np.int64)
    wi = np.asarray(inputs["wrong_input"]).astype(np.int64)
    tg = np.asarray(inputs["target_id"]).astype(np.int64)
    q_emb = np.asarray(inputs["q_emb"], dtype=f32)
    i_emb = np.asarray(inputs["i_emb"], dtype=f32)

    def W(name):
        return np.asarray(inputs[name], dtype=f32)

    common = {
        "i_emb": i_emb.astype(f16),
        "q_emb": q_emb.astype(f16),
        "wd_dram": _WD_ZERO,
        "erase_Wt": np.ascontiguousarray(W("erase_W").T).astype(f16),
        "add_Wt": np.ascontiguousarray(W("add_W").T).astype(f16),
        "key_Wt": np.ascontiguousarray(W("key_W").T).astype(f16),
        "erase_b_row": np.tile(W("erase_b").reshape(1, -1), (1, 4)).astype(f16),
        "add_b_row": np.tile(W("add_b").reshape(1, -1), (1, 4)).astype(f16),
        "rsum_Wt0": np.ascontiguousarray(W("rsum_W")[:, :VD].T).astype(f16),
        "rsum_Wt1": np.ascontiguousarray(W("rsum_W")[:, VD:].T).astype(f16),
        "wsum_Wt0": np.ascontiguousarray(W("wsum_W")[:, :VD].T).astype(f16),
        "wsum_Wt1": np.ascontiguousarray(W("wsum_W")[:, VD:].T).astype(f16),
        "rsum_b_col": W("rsum_b").reshape(-1, 1).astype(f32),
        "wsum_b_col": W("wsum_b").reshape(-1, 1).astype(f32),
        "succ_Wt": np.ascontiguousarray(W("succ_W").T).astype(f16),
        "fail_Wt": np.ascontiguousarray(W("fail_W").T).astype(f16),
        "diff_Wt": np.ascontiguousarray(W("diff_W").T).astype(f16),
        "succ_b": W("succ_b").reshape(1, 1).astype(f32),
        "fail_b": W("fail_b").reshape(1, 1).astype(f32),
        "diff_b": W("diff_b").reshape(1, 1).astype(f32),
        "rmem0": W("right_mem_init").astype(f16),
        "wmem0": W("wrong_mem_init").astype(f16),
        "ones_row": np.ones((1, 128), dtype=f16),
        "ones_col32": np.ones((128, 1), dtype=f32),
        "id128": np.eye(128, dtype=f16),
        "right_full": ri.astype(np.int32),
        "wrong_full": wi.astype(np.int32),
    }

    in_maps = []
    for core in range(NCORE):
        rows = slice(core * BL, (core + 1) * BL)
        # inter ids per (t, r): r<BL -> right, else wrong
        inter = np.empty((S, NR), dtype=np.int64)
        inter[:, :BL] = ri[rows].T
        inter[:, BL:] = wi[rows].T
        qid = inter - Q * (inter > Q)
        # both tables: t-major 128-row chunks (4 steps x 32 rows per chunk)
        idx_ip = inter.reshape(-1).reshape(NGQ, 128).T.astype(np.int32)
        flat_q = qid.reshape(-1)
        idx_q = flat_q.reshape(NGQ, 128).T.astype(np.int32)
        idx_t = tg[rows].reshape(BL, 1).astype(np.int32)
        in_maps.append({**common, "idx_i": np.ascontiguousarray(idx_ip),
                        "idx_q": np.ascontiguousarray(idx_q),
                        "idx_t": idx_t})
    return in_maps


def run_spmd(inputs, trace=False):
    nc = _get_program()
    in_maps = _host_inputs(inputs)
    res = run_bass_kernel_spmd(nc, in_maps, core_ids=list(range(NCORE)),
                               trace=trace)
    out = np.concatenate([res.results[i]["out"] for i in range(NCORE)], axis=0)
    return out.astype(np.float32), res


def kernel(**inputs):
    out, _ = run_spmd(inputs, trace=False)
    return out
